# revision 1
# baseline (speedup 1.0000x reference)
"""BoeNet greedy BFS rollout — Trainium2 Bass kernel (8 NeuronCores).

Fully data-parallel strategy (no collectives): each core owns 512 of the
4096 flattened positions and computes the FULL vocab row block for them.

Phase A (per core, 512 positions):
  embedding gather -> h0 = emb[tok] @ Wp + bp (f32r) -> greedy-gate tree.
  Gate z-values are computed directly as BROADCAST [128, 512] psum tiles
  (stationary = gate weight column replicated to 128 cols, host-prepared),
  so the compare (per-partition threshold, DVE) yields the broadcast 0/1
  mask directly — no row ops, no PE outer-products, no Act copies.
  Gates for the two children of node x fold through x itself via
  (Wc_side @ wg), so level-2 children are never materialized.
  Aggregation (masks commute with left-matmuls over columns):
    agg = h0 + wcs^T mA + (WcL wcs)^T mL + (WcR wcs)^T mR + bias outers
  with wcs = WcL+WcR, mA = e0*h0 + e1_0*n1_0 + e1_1*n1_1 (bf16 products,
  mA on Pool engine, mL/mR on DVE), mL/mR the level-2 masked sums.
  pooled = (h0 + agg_psum) * (1/count), 1/count = Exp(-Ln(count)) on Act.

Phase B: logits[pos, vocab] tiles [128 pos x 500 v]. Stationary = bf16
  pooledT slice (reused for 8 consecutive matmuls -> LDWEIGHTS mostly
  eliminated), moving = bf16 Wout streamed from HBM. Drains on DVE:
  (psum + bias_bcast) -> bf16, bias tiles DMA-broadcast from a [1, V]
  row. Output written bf16 [512, 32000] per core, upcast on host.
"""
import sys

for _p in ('/opt/trn_rl_repo', '/opt/pypackages'):
    if _p not in sys.path:
        sys.path.insert(0, _p)

import numpy as np

B, S, V, E, H = 8, 512, 32000, 512, 512
NPOS = B * S              # 4096 flattened positions
NCORES = 8
PC_POS = NPOS // NCORES   # 512 positions per core
MAX_DEPTH = 3
DEPTH_EMBED_SCALE = 0.01
SIB_SCALE = 1.0 / np.sqrt(H)

VW = 2000                 # vocab window (4 strips of 500)
NWIN = V // VW            # 16 windows
NSTR = 4                  # 500-col strips per window

_CACHE = {}


def _build():
    import concourse.bass as bass
    import concourse.bacc as bacc
    import concourse.tile as tile
    import concourse.mybir as mybir
    from concourse.masks import make_identity
    from contextlib import ExitStack

    F32 = mybir.dt.float32
    F32R = mybir.dt.float32r
    BF16 = mybir.dt.bfloat16
    I16 = mybir.dt.int16
    AF = mybir.ActivationFunctionType
    OP = mybir.AluOpType

    nc = bacc.Bacc("TRN2", target_bir_lowering=False, debug=False,
                   num_devices=NCORES)

    hemb_d = nc.dram_tensor("hemb", [128, 4 * E], F32, kind="ExternalInput")
    wp_d = nc.dram_tensor("wp", [E, H], F32, kind="ExternalInput")
    wc_d = nc.dram_tensor("wc", [H, 2 * H], F32, kind="ExternalInput")
    wgb_d = nc.dram_tensor("wgb", [H, 128], F32, kind="ExternalInput")
    wgcb_d = nc.dram_tensor("wgcb", [H, 256], F32, kind="ExternalInput")
    cols_d = nc.dram_tensor("cols", [128, 12], F32, kind="ExternalInput")
    b3_d = nc.dram_tensor("b3", [1, 3 * H], BF16, kind="ExternalInput")
    thr_d = nc.dram_tensor("thr", [128, 8], F32, kind="ExternalInput")
    wcsb_d = nc.dram_tensor("wcsb", [H, H], BF16, kind="ExternalInput")
    wab_d = nc.dram_tensor("wab", [H, H], BF16, kind="ExternalInput")
    wbb_d = nc.dram_tensor("wbb", [H, H], BF16, kind="ExternalInput")
    bout1_d = nc.dram_tensor("bout1", [1, V], F32, kind="ExternalInput")
    woutb_d = nc.dram_tensor("woutb", [H, V], BF16, kind="ExternalInput")
    logt_d = nc.dram_tensor("logt", [PC_POS, V], BF16, kind="ExternalOutput")

    def cp(out_ap, in_ap, bias=0.0):
        nc.scalar.activation(out_ap, in_ap, AF.Identity, bias=bias)

    with tile.TileContext(nc) as tc, ExitStack() as ctx:
        const = ctx.enter_context(tc.tile_pool(name="const", bufs=1))
        wpool = ctx.enter_context(tc.tile_pool(name="wpool", bufs=1))
        bpool = ctx.enter_context(tc.tile_pool(name="bpool", bufs=1))
        popool = ctx.enter_context(tc.tile_pool(name="popool", bufs=1))

        identity = const.tile([128, 128], F32, tag="ident", name="ident")
        make_identity(nc, identity[:])
        identity_r = const.tile([128, 128], F32R, tag="identr", name="identr")
        nc.scalar.activation(identity_r[:], identity[:],
                             mybir.ActivationFunctionType.Identity)
        ones_f32 = const.tile([1, 128], F32, tag="ones", name="ones")
        nc.vector.memset(ones_f32[:], 1.0)
        cols_sb = const.tile([128, 12], F32, tag="cols", name="cols")
        nc.sync.dma_start(cols_sb[:], cols_d[:])
        thr_sb = const.tile([128, 8], F32, tag="thr", name="thr")
        nc.sync.dma_start(thr_sb[:], thr_d[:])
        wgb_sb = const.tile([128, 4 * 128], F32R, tag="wgb", name="wgb")
        for hc in range(4):
            nc.sync.dma_start(wgb_sb[:, hc * 128:(hc + 1) * 128],
                              wgb_d[hc * 128:(hc + 1) * 128, :].bitcast(F32R))
        # wgcb layout in SBUF: per hc, [L-replica 128 | R-replica 128]
        wgcb_sb = const.tile([128, 4 * 256], F32R, tag="wgcb", name="wgcb")
        for hc in range(4):
            nc.sync.dma_start(wgcb_sb[:, hc * 256:(hc + 1) * 256],
                              wgcb_d[hc * 128:(hc + 1) * 128, :].bitcast(F32R))
        b3_sb = const.tile([1, 3 * H], BF16, tag="b3", name="b3")
        nc.sync.dma_start(b3_sb[:], b3_d[:])
        wcsb_sb, wab_sb, wbb_sb = [], [], []
        for nm, dt_, lst in (("wcsb", wcsb_d, wcsb_sb), ("wab", wab_d, wab_sb),
                             ("wbb", wbb_d, wbb_sb)):
            for hc in range(4):
                t = const.tile([128, H], BF16, tag=f"{nm}{hc}", name=f"{nm}{hc}")
                nc.sync.dma_start(t[:], dt_[hc * 128:(hc + 1) * 128, :])
                lst.append(t)

        # pooledT (bf16, [H, pos]) persists across phase A -> B
        pooT = [popool.tile([128, PC_POS], BF16, tag=f"pooT{jc}", name=f"pooT{jc}")
                for jc in range(4)]

        # phase-B weight stream + bias-broadcast tiles (prefetch w0, w1)
        def load_win(w):
            ts_ = []
            for hc in range(4):
                t = wpool.tile([128, VW], BF16, tag=f"ww{hc}",
                               name=f"ww{w}_{hc}", bufs=2)
                nc.sync.dma_start(
                    t[:], woutb_d[hc * 128:(hc + 1) * 128, w * VW:(w + 1) * VW])
                ts_.append(t)
            bts = []
            for s_ in range(NSTR):
                bt = bpool.tile([128, 500], F32, tag=f"bb{s_}",
                                name=f"bb{w}_{s_}", bufs=2)
                c0 = w * VW + s_ * 500
                nc.sync.dma_start(
                    bt[:], bout1_d[0:1, c0:c0 + 500].to_broadcast((128, 500)))
                bts.append(bt)
            return ts_, bts

        # ---------------- Phase A ----------------
        with ExitStack() as actx:
            apool = actx.enter_context(tc.tile_pool(name="apool", bufs=1))
            npool = actx.enter_context(tc.tile_pool(name="npool", bufs=1))
            wcpool = actx.enter_context(tc.tile_pool(name="wcpool", bufs=1))
            rpool = actx.enter_context(tc.tile_pool(name="rpool", bufs=1))
            ebpool = actx.enter_context(tc.tile_pool(name="ebpool", bufs=1))
            mpool = actx.enter_context(tc.tile_pool(name="mpool", bufs=1))
            scr = actx.enter_context(tc.tile_pool(name="scr", bufs=2, space="PSUM"))

            # host-gathered embedding rows: [128, 4x512]
            # (row i -> partition i%128, chunk i//128), chunked DMAs so the
            # first transposes start as soon as chunk 0 lands
            gat_all = apool.tile([128, 4 * 512], F32, tag="gat", name="gat")
            for pc in range(4):
                nc.sync.dma_start(gat_all[:, pc * 512:(pc + 1) * 512],
                                  hemb_d[:, pc * 512:(pc + 1) * 512])
            gat = [gat_all[:, pc * 512:(pc + 1) * 512] for pc in range(4)]

            wp_sb, wc_sb = [], []
            for ec in range(4):
                t = npool.tile([128, 512], F32R, tag=f"wp{ec}", name=f"wp{ec}")
                nc.sync.dma_start(t[:], wp_d[ec * 128:(ec + 1) * 128, :].bitcast(F32R))
                wp_sb.append(t)
            for hc in range(4):
                t = wcpool.tile([128, 1024], F32R, tag=f"wc{hc}", name=f"wc{hc}")
                nc.sync.dma_start(t[:], wc_d[hc * 128:(hc + 1) * 128, :].bitcast(F32R))
                wc_sb.append(t)

            # phase-B prefetch AFTER the phase-A critical loads
            win_tiles = {0: load_win(0), 1: load_win(1)}

            with nc.allow_low_precision(reason="f32r/bf16 matmul inputs"):
                # transpose gathered embeddings -> hembT[ec] = [128 e, 512 pos]
                # (4 transposes share one psum tile -> ONE act drain per ec)
                hembT = [npool.tile([128, 512], F32R, tag=f"hembT{ec}",
                                    name=f"hembT{ec}") for ec in range(4)]
                for ec in range(4):
                    tp = scr.tile([128, 512], F32, tag="s", name=f"tp{ec}")
                    for pc in range(4):
                        nc.tensor.transpose(tp[:, pc * 128:(pc + 1) * 128],
                                            gat[pc][:, ec * 128:(ec + 1) * 128].opt(),
                                            identity[:])
                    cp(hembT[ec][:], tp[:])

                # h0 = emb@Wp + bp (bias via act copy)
                h0_sb = []
                for hc in range(4):
                    ps = scr.tile([128, 512], F32, tag="s", name="h0ps")
                    for ec in range(4):
                        nc.tensor.matmul(ps[:], wp_sb[ec][:, hc * 128:(hc + 1) * 128],
                                         hembT[ec][:], start=(ec == 0), stop=(ec == 3))
                    t = npool.tile([128, 512], F32R, tag=f"h0_{hc}", name=f"h0_{hc}")
                    nc.scalar.activation(t[:], ps[:], AF.Identity,
                                         bias=cols_sb[:, hc:hc + 1])
                    h0_sb.append(t)

                # Gates with PSUM-resident masks: z matmul -> in-place
                # compare (mask stays in PSUM so DVE mask products use the
                # separate SBUF+PSUM read ports, ~2x faster than SBUF+SBUF).
                # Pool engine cannot read PSUM, so it works from bf16 SBUF
                # copies made by the (otherwise idle) Act engine.
                mA, mL, mR, tps = [], [], [], []
                with tc.tile_pool(name="zmask", bufs=6, space="PSUM") as zmask:
                    def gate_z(node, wsel, thr_col, nm):
                        zps = zmask.tile([128, 512], F32, tag="z", name=f"z{nm}")
                        for hc in range(4):
                            nc.tensor.matmul(zps[:], wsel(hc), node[hc][:],
                                             start=(hc == 0), stop=(hc == 3))
                        nc.vector.tensor_scalar(zps[:], zps[:],
                                                thr_sb[:, thr_col:thr_col + 1],
                                                None, OP.is_gt)
                        return zps

                    e0p = gate_z(h0_sb, lambda hc: wgb_sb[:, hc * 128:(hc + 1) * 128],
                                 0, "0")
                    e0sb = ebpool.tile([128, 512], BF16, tag="e0sb", name="e0sb")
                    cp(e0sb[:], e0p[:])
                    # mA starts as e0*h0 on Pool (SBUF mask) freeing e0p early
                    for hc in range(4):
                        a = mpool.tile([128, 512], BF16, tag=f"mA{hc}", name=f"mA{hc}")
                        nc.gpsimd.tensor_tensor(a[:], h0_sb[hc][:], e0sb[:], op=OP.mult)
                        mA.append(a)

                    # children level 1 (f32r: they feed the level-2 gates)
                    n1 = [[], []]
                    for side in (0, 1):
                        for jc2 in range(4):
                            jq = side * 4 + jc2
                            ps = scr.tile([128, 512], F32, tag="s", name="chps")
                            for hc in range(4):
                                nc.tensor.matmul(ps[:], wc_sb[hc][:, jq * 128:(jq + 1) * 128],
                                                 h0_sb[hc][:], start=(hc == 0), stop=(hc == 3))
                            t = npool.tile([128, 512], F32R, tag=f"n1_{side}_{jc2}",
                                           name=f"n1_{side}_{jc2}")
                            nc.scalar.activation(t[:], ps[:], AF.Identity,
                                                 bias=cols_sb[:, 4 + side * 4 + jc2:
                                                              5 + side * 4 + jc2])
                            n1[side].append(t)

                    # level-1 gates (folded from h0); parent-mask in place
                    e1p, e1sb = [], []
                    for side in (0, 1):
                        zp1 = gate_z(h0_sb,
                                     lambda hc: wgcb_sb[:, hc * 256 + side * 128:
                                                        hc * 256 + side * 128 + 128],
                                     1 + side, f"1_{side}")
                        nc.vector.tensor_tensor(zp1[:], zp1[:], e0sb[:], op=OP.mult)
                        sb = ebpool.tile([128, 512], BF16, tag=f"e1sb{side}",
                                         name=f"e1sb{side}")
                        cp(sb[:], zp1[:])
                        e1p.append(zp1)
                        e1sb.append(sb)

                    # mA += e1_j * n1_j on Pool via the SBUF mask copies
                    for hc in range(4):
                        t1 = mpool.tile([128, 512], BF16, tag=f"tA{hc}", name=f"tA{hc}")
                        nc.gpsimd.tensor_tensor(t1[:], n1[0][hc][:], e1sb[0][:], op=OP.mult)
                        nc.gpsimd.tensor_tensor(mA[hc][:], mA[hc][:], t1[:], op=OP.add)
                        nc.gpsimd.tensor_tensor(t1[:], n1[1][hc][:], e1sb[1][:], op=OP.mult)
                        nc.gpsimd.tensor_tensor(mA[hc][:], mA[hc][:], t1[:], op=OP.add)

                    # level-2 gates; masks stay in PSUM (DVE consumers)
                    e2p = []
                    er2 = []
                    for j in (0, 1):
                        for side in (0, 1):
                            zp2 = gate_z(n1[j],
                                         lambda hc: wgcb_sb[:, hc * 256 + side * 128:
                                                            hc * 256 + side * 128 + 128],
                                         3 + side, f"2_{2 * j + side}")
                            nc.vector.tensor_tensor(zp2[:], zp2[:], e1sb[j][:], op=OP.mult)
                            r = rpool.tile([1, 512], BF16, tag=f"er2_{2*j+side}",
                                           name=f"er2_{2*j+side}")
                            cp(r[:], zp2[0:1, :])
                            e2p.append(zp2)
                            er2.append(r)

                    # mL/mR on DVE: SBUF node x PSUM mask, psum scratch for t
                    for hc in range(4):
                        l_ = mpool.tile([128, 512], BF16, tag=f"mL{hc}", name=f"mL{hc}")
                        t2 = scr.tile([128, 512], F32, tag="s", name=f"tL{hc}")
                        nc.vector.tensor_tensor(l_[:], n1[0][hc][:], e2p[0][:], op=OP.mult)
                        nc.vector.tensor_tensor(t2[:], n1[1][hc][:], e2p[2][:], op=OP.mult)
                        nc.vector.tensor_tensor(l_[:], l_[:], t2[:], op=OP.add)
                        mL.append(l_)
                    for hc in range(4):
                        r_ = mpool.tile([128, 512], BF16, tag=f"mR{hc}", name=f"mR{hc}")
                        t3 = scr.tile([128, 512], F32, tag="s", name=f"tR{hc}")
                        nc.vector.tensor_tensor(r_[:], n1[0][hc][:], e2p[1][:], op=OP.mult)
                        nc.vector.tensor_tensor(t3[:], n1[1][hc][:], e2p[3][:], op=OP.mult)
                        nc.vector.tensor_tensor(r_[:], r_[:], t3[:], op=OP.add)
                        mR.append(r_)

                e1 = e1sb
                # expansion-count rows from mask row 0 (exact ints in bf16)
                s1 = rpool.tile([1, 512], F32, tag="s1", name="s1")
                nc.vector.tensor_tensor(s1[:], e0sb[0:1, :], e1[0][0:1, :], op=OP.add)
                nc.vector.tensor_tensor(s1[:], s1[:], e1[1][0:1, :], op=OP.add)
                sL = rpool.tile([1, 512], BF16, tag="sL", name="sL")
                nc.vector.tensor_tensor(sL[:], er2[0][:], er2[2][:], op=OP.add)
                sR = rpool.tile([1, 512], BF16, tag="sR", name="sR")
                nc.vector.tensor_tensor(sR[:], er2[1][:], er2[3][:], op=OP.add)
                esum = rpool.tile([1, 512], F32, tag="esum", name="esum")
                nc.vector.tensor_tensor(esum[:], sL[:], sR[:], op=OP.add)
                nc.vector.tensor_tensor(esum[:], esum[:], s1[:], op=OP.add)
                esb = rpool.tile([1, 512], BF16, tag="esb", name="esb")
                nc.vector.tensor_copy(esb[:], esum[:])
                # count = 1 + 2*esum; 1/count = Exp(-Ln(count)) on Act
                cnt = rpool.tile([1, 512], F32, tag="cnt", name="cnt")
                nc.vector.tensor_scalar(cnt[:], esum[:], 2.0, 1.0, OP.mult, OP.add)
                lncnt = rpool.tile([1, 512], F32, tag="lncnt", name="lncnt")
                nc.scalar.activation(lncnt[:], cnt[:], AF.Ln)
                recipr = rpool.tile([1, 512], F32R, tag="recipr", name="recipr")
                nc.scalar.activation(recipr[:], lncnt[:], AF.Exp, scale=-1.0)
                rbp = scr.tile([128, 512], F32, tag="s", name="rbp")
                nc.tensor.matmul(rbp[:], ones_f32[0:1, :].bitcast(F32R), recipr[:],
                                 start=True, stop=True)
                rb_sb = ebpool.tile([128, 512], F32, tag="rb", name="rb")
                cp(rb_sb[:], rbp[:])

                # agg accumulation in PSUM (banks freed by zmask exit)
                aggp = actx.enter_context(tc.tile_pool(name="aggp", bufs=4,
                                                       space="PSUM"))
                agg_ps = []
                brow = [esb, sL, sR]
                for jc in range(4):
                    ap_ = aggp.tile([128, 512], F32, tag="agg", name=f"agg{jc}")
                    for hc in range(4):
                        nc.tensor.matmul(ap_[:], wab_sb[hc][:, jc * 128:(jc + 1) * 128],
                                         mL[hc][:], start=(hc == 0), stop=False)
                    for hc in range(4):
                        nc.tensor.matmul(ap_[:], wbb_sb[hc][:, jc * 128:(jc + 1) * 128],
                                         mR[hc][:], start=False, stop=False)
                    for r in range(3):
                        nc.tensor.matmul(ap_[:],
                                         b3_sb[0:1, r * H + jc * 128:
                                               r * H + (jc + 1) * 128],
                                         brow[r][:], start=False, stop=False)
                    for hc in range(4):
                        nc.tensor.matmul(ap_[:], wcsb_sb[hc][:, jc * 128:(jc + 1) * 128],
                                         mA[hc][:], start=False, stop=False)
                    # + h0 via identity stationary (frees the DVE psum-add)
                    nc.tensor.matmul(ap_[:], identity_r[:],
                                     h0_sb[jc][:], start=False, stop=True)
                    agg_ps.append(ap_)

                # pooled = agg * recip -> bf16 (h0 already in agg; Pool
                # cannot read PSUM so all four run on DVE)
                for jc in range(4):
                    nc.vector.tensor_tensor(pooT[jc][:], agg_ps[jc][:],
                                            rb_sb[:], op=OP.mult)

        # ---------------- Phase B ----------------
        with ExitStack() as bctx:
            stp = bctx.enter_context(tc.tile_pool(name="stp", bufs=4))
            mmp = bctx.enter_context(tc.tile_pool(name="mmp", bufs=8, space="PSUM"))

            with nc.allow_low_precision(reason="bf16 matmul inputs"):
                for w in range(NWIN):
                    wt, bts = win_tiles.pop(w)
                    for pc in range(4):
                        stg = stp.tile([128, VW], BF16, tag="stage",
                                       name=f"stg{w}_{pc}")
                        for s_ in range(NSTR):
                            ps = mmp.tile([128, 500], F32, tag="mm",
                                          name=f"mm{w}_{pc}_{s_}", bufs=8)
                            for hc in range(4):
                                nc.tensor.matmul(
                                    ps[:],
                                    pooT[hc][:, pc * 128:(pc + 1) * 128],
                                    wt[hc][:, s_ * 500:(s_ + 1) * 500],
                                    start=(hc == 0), stop=(hc == 3))
                            nc.vector.tensor_tensor(
                                stg[:, s_ * 500:(s_ + 1) * 500], ps[:],
                                bts[s_][:], op=OP.add)
                        nc.sync.dma_start(
                            logt_d[pc * 128:(pc + 1) * 128, w * VW:(w + 1) * VW],
                            stg[:])
                    if w + 2 < NWIN:
                        win_tiles[w + 2] = load_win(w + 2)

    nc.compile()
    return nc


def _get_nc():
    if "nc" not in _CACHE:
        _CACHE["nc"] = _build()
    return _CACHE["nc"]


def _prep_inputs(tokens, emb, Wp, bp, Wc, bc, Wg, bg, dep, sib, Wout, bout):
    import ml_dtypes
    BF = ml_dtypes.bfloat16

    tokens = np.asarray(tokens).astype(np.int32).reshape(-1)
    emb = np.ascontiguousarray(np.asarray(emb, dtype=np.float32))
    Wp = np.ascontiguousarray(np.asarray(Wp, dtype=np.float32))
    bp = np.asarray(bp, dtype=np.float64).reshape(-1)
    Wc = np.ascontiguousarray(np.asarray(Wc, dtype=np.float32))
    bc = np.asarray(bc, dtype=np.float64).reshape(-1)
    Wg = np.ascontiguousarray(np.asarray(Wg, dtype=np.float32))
    bg = np.asarray(bg, dtype=np.float64).reshape(-1)
    dep = np.asarray(dep, dtype=np.float64)
    sib = np.asarray(sib, dtype=np.float64)
    Wout = np.asarray(Wout, dtype=np.float32)
    bout = np.asarray(bout, dtype=np.float32).reshape(-1)

    WcL = Wc[:, :H].astype(np.float64)
    WcR = Wc[:, H:].astype(np.float64)
    wg64 = Wg[:, 0].astype(np.float64)
    wcs = WcL + WcR
    biasL = bc[:H] + SIB_SCALE * sib[0]
    biasR = bc[H:] + SIB_SCALE * sib[1]
    bsum = biasL + biasR

    # per-partition bias columns for act-engine copies (bp, biasL, biasR)
    cols = np.ascontiguousarray(np.concatenate(
        [bp.reshape(4, 128).T, biasL.reshape(4, 128).T, biasR.reshape(4, 128).T],
        axis=1)).astype(np.float32)

    # gate weights replicated across 128 output columns (broadcast-z trick)
    wgb = np.repeat(Wg.astype(np.float32), 128, axis=1)          # [H, 128]
    wgcL = (WcL @ wg64).astype(np.float32)
    wgcR = (WcR @ wg64).astype(np.float32)
    wgcb = np.concatenate([np.repeat(wgcL[:, None], 128, axis=1),
                           np.repeat(wgcR[:, None], 128, axis=1)], axis=1)

    # thresholds (replicated to 128 partitions):
    # col 0=root, 1=d1 L, 2=d1 R, 3=d2 L, 4=d2 R
    cd = DEPTH_EMBED_SCALE * (dep @ wg64) + bg[0]
    cL = biasL @ wg64
    cR = biasR @ wg64
    thr = np.zeros((1, 8), np.float64)
    thr[0, 0] = -cd[0]
    thr[0, 1] = -(cL + cd[1])
    thr[0, 2] = -(cR + cd[1])
    thr[0, 3] = -(cL + cd[2])
    thr[0, 4] = -(cR + cd[2])
    thr = np.repeat(thr.astype(np.float32), 128, axis=0)

    # agg weights (bf16): wcs, WcL@wcs, WcR@wcs; bias rows b3 [1, 3H]
    wcsb = wcs.astype(np.float32).astype(BF)
    wab = (WcL @ wcs).astype(np.float32).astype(BF)
    wbb = (WcR @ wcs).astype(np.float32).astype(BF)
    b3 = np.concatenate([bsum, biasL @ wcs, biasR @ wcs]).reshape(1, 3 * H) \
        .astype(np.float32).astype(BF)

    bout1 = np.ascontiguousarray(bout.reshape(1, V))
    woutb = np.ascontiguousarray(Wout.astype(BF))

    in_maps = []
    for c in range(NCORES):
        tk = tokens[c * PC_POS:(c + 1) * PC_POS]
        hemb = np.ascontiguousarray(
            emb[tk].reshape(4, 128, E).transpose(1, 0, 2).reshape(128, 4 * E))
        in_maps.append({
            "hemb": hemb, "wp": Wp, "wc": Wc,
            "wgb": np.ascontiguousarray(wgb),
            "wgcb": np.ascontiguousarray(wgcb),
            "cols": cols, "b3": np.ascontiguousarray(b3),
            "thr": np.ascontiguousarray(thr),
            "wcsb": wcsb, "wab": wab, "wbb": wbb,
            "bout1": bout1, "woutb": woutb,
        })
    return in_maps


def _assemble(res):
    parts = [np.asarray(res.results[c]["logt"]).astype(np.float32)
             for c in range(NCORES)]
    full = np.concatenate(parts, axis=0)        # [NPOS, V]
    return full.reshape(B, S, V)


def _enable_ldw_opt_once():
    # ldw-opt rejects the bf16/K=1 stationaries this kernel uses — keep the
    # compiler default (off). bf16 weight loads are cheap without dedup.
    return


def kernel(**inputs) -> np.ndarray:
    from concourse.bass_utils import run_bass_kernel_spmd
    nc = _get_nc()
    in_maps = _prep_inputs(**inputs)
    res = run_bass_kernel_spmd(nc, in_maps, list(range(NCORES)))
    return _assemble(res)



# revision 41
# speedup vs baseline: 1.0623x; 1.0623x over previous
"""BoeNet greedy BFS rollout — Trainium2 Bass kernel (8 NeuronCores), v2.

Fully data-parallel: each core owns 512 of the 4096 flattened positions and
computes the full vocab row block for them.  No collectives.

v2 vs v1 (322us baseline):
  * Host ships the gathered embedding rows already TRANSPOSED (hembT
    [E, pos]), both f32 (for gates) and bf16 (for values) — no on-device
    transposes, and phase A starts as soon as the first 256KB chunk lands.
  * All 7 tree gates are folded to embedding space on the host
    (v = Wp Wc... wg), so ONE accumulated matmul (stationary [128,8])
    produces every gate z as rows [8, 512].  Thresholding is one DVE
    tensor_scalar; the ancestor-AND + broadcast to [128,512] is done with
    tiny K=7 matmuls against 0/1 selector columns followed by a compare.
  * h0/n1 values in bf16 (gates no longer read them), halving weight DMA
    and enabling 16-bit DVE throughput for the mask products.
  * Tiny/critical DMAs are issued first; Wout window prefetch is deferred.
  * bout is added on the host: phase-B drains become pure psum->bf16
    copies, alternated between DVE and Act.  16MB of bias-broadcast DMA
    traffic disappears.
  * Phase-B window 0 runs hc-outer so its first matmuls start as soon as
    pooT[0] lands, overlapping the agg tail.
"""
import sys

for _p in ('/opt/trn_rl_repo', '/opt/pypackages'):
    if _p not in sys.path:
        sys.path.insert(0, _p)

import numpy as np

B, S, V, E, H = 8, 512, 32000, 512, 512
NPOS = B * S              # 4096 flattened positions
NCORES = 8
PC_POS = NPOS // NCORES   # 512 positions per core
MAX_DEPTH = 3
DEPTH_EMBED_SCALE = 0.01
SIB_SCALE = 1.0 / np.sqrt(H)

VW = 2000                 # vocab window (4 strips of 500)
NWIN = V // VW            # 16 windows
NSTR = 4                  # 500-col strips per window

# masks: e0, e1L, e1R, e2LL, e2LR, e2RL, e2RR
MASK_SEL = [(0,), (0, 1), (0, 2), (0, 1, 3), (0, 1, 4), (0, 2, 5), (0, 2, 6)]

_CACHE = {}
DEBUG_DUMP = False


def _build():
    import concourse.bass as bass
    import concourse.bacc as bacc
    import concourse.tile as tile
    import concourse.mybir as mybir
    from concourse.masks import make_identity
    from contextlib import ExitStack

    F32 = mybir.dt.float32
    F32R = mybir.dt.float32r
    BF16 = mybir.dt.bfloat16
    AF = mybir.ActivationFunctionType
    OP = mybir.AluOpType

    nc = bacc.Bacc("TRN2", target_bir_lowering=False, debug=False,
                   num_devices=NCORES)

    # --- dram inputs (order here is irrelevant; DMA issue order matters) ---
    wz_d = nc.dram_tensor("wz", [128, 32], F32, kind="ExternalInput")
    thr7_d = nc.dram_tensor("thr7", [8, 1], F32, kind="ExternalInput")
    selb_d = nc.dram_tensor("selb", [7, 7 * 128], BF16, kind="ExternalInput")
    sel7_d = nc.dram_tensor("sel7", [7, 8], BF16, kind="ExternalInput")
    cthr_d = nc.dram_tensor("cthr", [8, 1], F32, kind="ExternalInput")
    bw7_d = nc.dram_tensor("bw7", [7, H], BF16, kind="ExternalInput")
    cols_d = nc.dram_tensor("cols", [128, 12], F32, kind="ExternalInput")
    hembt_d = nc.dram_tensor("hembt", [128, 4 * 512], F32, kind="ExternalInput")
    hembtb_d = nc.dram_tensor("hembtb", [128, 4 * 512], BF16,
                              kind="ExternalInput")
    wpb_d = nc.dram_tensor("wpb", [E, H], BF16, kind="ExternalInput")
    wcb_d = nc.dram_tensor("wcb", [H, 2 * H], BF16, kind="ExternalInput")
    wcsb_d = nc.dram_tensor("wcsb", [H, H], BF16, kind="ExternalInput")
    wab_d = nc.dram_tensor("wab", [H, H], BF16, kind="ExternalInput")
    wbb_d = nc.dram_tensor("wbb", [H, H], BF16, kind="ExternalInput")
    woutb_d = nc.dram_tensor("woutb", [H, V], BF16, kind="ExternalInput")
    logt_d = nc.dram_tensor("logt", [PC_POS, V], BF16, kind="ExternalOutput")
    if DEBUG_DUMP:
        dzr_d = nc.dram_tensor("dzr", [8, 512], BF16, kind="ExternalOutput")
        dmask_d = nc.dram_tensor("dmask", [128, 7 * 512], BF16,
                                 kind="ExternalOutput")
        dh0_d = nc.dram_tensor("dh0", [128, 4 * 512], BF16,
                               kind="ExternalOutput")
        dpoo_d = nc.dram_tensor("dpoo", [128, 4 * 512], BF16,
                                kind="ExternalOutput")
        dn1_d = nc.dram_tensor("dn1", [128, 8 * 512], BF16,
                               kind="ExternalOutput")
        dprod_d = nc.dram_tensor("dprod", [128, 12 * 512], BF16,
                                 kind="ExternalOutput")
        drow_d = nc.dram_tensor("drow", [128, 4 * 512], mybir.dt.float32,
                                kind="ExternalOutput")
        dagg_d = nc.dram_tensor("dagg", [128, 4 * 512], mybir.dt.float32,
                                kind="ExternalOutput")

    with tile.TileContext(nc) as tc, ExitStack() as ctx:
        const = ctx.enter_context(tc.tile_pool(name="const", bufs=1))
        wpool = ctx.enter_context(tc.tile_pool(name="wpool", bufs=1))
        popool = ctx.enter_context(tc.tile_pool(name="popool", bufs=1))

        # ---- tiny consts first (fast DMAs, unblock the gate pipeline) ----
        wz_sb = const.tile([128, 32], F32R, tag="wz", name="wz")
        nc.sync.dma_start(wz_sb[:], wz_d[:].bitcast(F32R))
        thr7_sb = const.tile([8, 1], F32, tag="thr7", name="thr7")
        nc.sync.dma_start(thr7_sb[:], thr7_d[:])
        selb_sb = const.tile([7, 7 * 128], BF16, tag="selb", name="selb")
        nc.sync.dma_start(selb_sb[:], selb_d[:])
        cols_sb = const.tile([128, 12], F32, tag="cols", name="cols")
        nc.sync.dma_start(cols_sb[:], cols_d[:])
        sel7_sb = const.tile([7, 8], BF16, tag="sel7", name="sel7")
        nc.sync.dma_start(sel7_sb[:], sel7_d[:])
        cthr_sb = const.tile([8, 1], F32, tag="cthr", name="cthr")
        nc.sync.dma_start(cthr_sb[:], cthr_d[:])
        bw7_sb = const.tile([7, H], BF16, tag="bw7", name="bw7")
        nc.sync.dma_start(bw7_sb[:], bw7_d[:])
        ones_f32 = const.tile([1, 128], F32, tag="ones", name="ones")
        nc.vector.memset(ones_f32[:], 1.0)
        identf = const.tile([128, 128], F32, tag="identf", name="identf")
        make_identity(nc, identf[:])
        identb = const.tile([128, 128], BF16, tag="identb", name="identb")
        nc.scalar.activation(identb[:], identf[:], AF.Identity)

        # ---- critical big loads, in priority order ----
        npool = ctx.enter_context(tc.tile_pool(name="npool", bufs=1))
        hembT = []
        for ec in range(4):
            t = npool.tile([128, 512], F32R, tag=f"hembT{ec}", name=f"hembT{ec}")
            nc.sync.dma_start(t[:], hembt_d[:, ec * 512:(ec + 1) * 512]
                              .bitcast(F32R))
            hembT.append(t)
        hembTb = []
        for ec in range(4):
            t = npool.tile([128, 512], BF16, tag=f"hembTb{ec}", name=f"hembTb{ec}")
            nc.sync.dma_start(t[:], hembtb_d[:, ec * 512:(ec + 1) * 512])
            hembTb.append(t)
        wpb_sb = []
        for ec in range(4):
            t = npool.tile([128, 512], BF16, tag=f"wpb{ec}", name=f"wpb{ec}")
            nc.sync.dma_start(t[:], wpb_d[ec * 128:(ec + 1) * 128, :])
            wpb_sb.append(t)
        wcb_sb = []
        for hc in range(4):
            t = npool.tile([128, 1024], BF16, tag=f"wcb{hc}", name=f"wcb{hc}")
            nc.sync.dma_start(t[:], wcb_d[hc * 128:(hc + 1) * 128, :])
            wcb_sb.append(t)
        wcsb_sb, wab_sb, wbb_sb = [], [], []
        for nm, dt_, lst in (("wcsb", wcsb_d, wcsb_sb), ("wab", wab_d, wab_sb),
                             ("wbb", wbb_d, wbb_sb)):
            for hc in range(4):
                t = npool.tile([128, H], BF16, tag=f"{nm}{hc}", name=f"{nm}{hc}")
                nc.sync.dma_start(t[:], dt_[hc * 128:(hc + 1) * 128, :])
                lst.append(t)

        # pooledT (bf16, [H, pos]) persists across phase A -> B
        pooT = [popool.tile([128, PC_POS], BF16, tag=f"pooT{jc}", name=f"pooT{jc}")
                for jc in range(4)]

        # phase-B weight window stream (issued after phase-A criticals)
        def load_win(w):
            ts_ = []
            for hc in range(4):
                t = wpool.tile([128, VW], BF16, tag=f"ww{hc}",
                               name=f"ww{w}_{hc}", bufs=2)
                nc.sync.dma_start(
                    t[:], woutb_d[hc * 128:(hc + 1) * 128, w * VW:(w + 1) * VW])
                ts_.append(t)
            return ts_

        # ---------------- Phase A ----------------
        with ExitStack() as actx:
            apool = actx.enter_context(tc.tile_pool(name="apool", bufs=1))
            scr = actx.enter_context(tc.tile_pool(name="scr", bufs=2,
                                                  space="PSUM"))

            win_tiles = {0: load_win(0), 1: load_win(1)}

            with nc.allow_low_precision(reason="bf16/f32r matmul inputs"):
                # gate z rows [8, 512] — all 7 gates in one accumulation.
                # zmask psum pool closes before aggp opens (bank budget).
                masks = []
                with tc.tile_pool(name="zmask", bufs=2, space="PSUM") as zmask:
                    zp = zmask.tile([8, 512], F32, tag="zp", name="zp",
                                    bufs=1)
                    for ec in range(4):
                        nc.tensor.matmul(zp[:], wz_sb[:, ec * 8:(ec + 1) * 8],
                                         hembT[ec][:], start=(ec == 0),
                                         stop=(ec == 3))
                    zr01 = apool.tile([8, 512], BF16, tag="zr01", name="zr01")
                    nc.vector.tensor_scalar(zr01[:], zp[:], thr7_sb[:, 0:1],
                                            None, OP.is_gt)

                    # ancestor-AND + partition broadcast via K=7 matmuls:
                    # psum = (# required gates that fired), mask = psum > n-0.5
                    for m in range(7):
                        ms = zmask.tile([128, 512], F32, tag="mscr",
                                        name=f"ms{m}")
                        nc.tensor.matmul(ms[:],
                                         selb_sb[:, m * 128:(m + 1) * 128],
                                         zr01[0:7, :], start=True, stop=True)
                        mk = apool.tile([128, 512], BF16, tag=f"mask{m}",
                                        name=f"mask{m}")
                        nc.vector.tensor_scalar(mk[:], ms[:],
                                                float(len(MASK_SEL[m])) - 0.5,
                                                None, OP.is_gt)
                        masks.append(mk)
                    # stacked masks [7,512]: all 7 AND-counts as rows, then
                    # one compare with per-row thresholds
                    mr_ps = zmask.tile([8, 512], F32, tag="mr", name="mr",
                                       bufs=1)
                    nc.tensor.matmul(mr_ps[0:7, :], sel7_sb[:, 0:7],
                                     zr01[0:7, :], start=True, stop=True)
                    mrows = apool.tile([7, 512], BF16, tag="mrows",
                                       name="mrows")
                    nc.vector.tensor_scalar(mrows[:], mr_ps[0:7, :],
                                            cthr_sb[0:7, 0:1], None, OP.is_gt)
                e0m, e1Lm, e1Rm, eLLm, eLRm, eRLm, eRRm = masks
                if DEBUG_DUMP:
                    nc.sync.dma_start(dzr_d[:], zr01[:])
                    for m in range(7):
                        nc.sync.dma_start(dmask_d[:, m * 512:(m + 1) * 512],
                                          masks[m][:])

                # h0 = embT @ Wp + bp   (bf16 values; gates don't read h0)
                h0b = []
                for hc in range(4):
                    ps = scr.tile([128, 512], F32, tag="s", name=f"h0ps{hc}")
                    for ec in range(4):
                        nc.tensor.matmul(ps[:],
                                         wpb_sb[ec][:, hc * 128:(hc + 1) * 128],
                                         hembTb[ec][:], start=(ec == 0),
                                         stop=(ec == 3))
                    t = apool.tile([128, 512], BF16, tag=f"h0_{hc}",
                                   name=f"h0_{hc}")
                    nc.scalar.activation(t[:], ps[:], AF.Identity,
                                         bias=cols_sb[:, hc:hc + 1])
                    h0b.append(t)
                if DEBUG_DUMP:
                    for hc in range(4):
                        nc.sync.dma_start(dh0_d[:, hc * 512:(hc + 1) * 512],
                                          h0b[hc][:])

                # level-1 children values (bf16)
                n1b = [[], []]
                for side in (0, 1):
                    for jc2 in range(4):
                        jq = side * 4 + jc2
                        ps = scr.tile([128, 512], F32, tag="s", name=f"chps{jq}")
                        for hc in range(4):
                            nc.tensor.matmul(
                                ps[:], wcb_sb[hc][:, jq * 128:(jq + 1) * 128],
                                h0b[hc][:], start=(hc == 0), stop=(hc == 3))
                        t = apool.tile([128, 512], BF16, tag=f"n1_{side}_{jc2}",
                                       name=f"n1_{side}_{jc2}")
                        nc.scalar.activation(t[:], ps[:], AF.Identity,
                                             bias=cols_sb[:, 4 + jq:5 + jq])
                        n1b[side].append(t)

                # masked sums (all-bf16 SBUF → 16-bit DVE/Pool throughput)
                mA, mL, mR = [], [], []
                for hc in range(4):
                    a = apool.tile([128, 512], BF16, tag=f"mA{hc}", name=f"mA{hc}")
                    t1 = apool.tile([128, 512], BF16, tag=f"tA{hc}", name=f"tA{hc}")
                    nc.gpsimd.tensor_tensor(a[:], h0b[hc][:], e0m[:], op=OP.mult)
                    nc.gpsimd.tensor_tensor(t1[:], n1b[0][hc][:], e1Lm[:],
                                            op=OP.mult)
                    nc.gpsimd.tensor_tensor(a[:], a[:], t1[:], op=OP.add)
                    nc.gpsimd.tensor_tensor(t1[:], n1b[1][hc][:], e1Rm[:],
                                            op=OP.mult)
                    nc.gpsimd.tensor_tensor(a[:], a[:], t1[:], op=OP.add)
                    mA.append(a)
                    l_ = apool.tile([128, 512], BF16, tag=f"mL{hc}", name=f"mL{hc}")
                    t2 = apool.tile([128, 512], BF16, tag=f"tL{hc}", name=f"tL{hc}")
                    nc.vector.tensor_tensor(l_[:], n1b[0][hc][:], eLLm[:],
                                            op=OP.mult)
                    nc.vector.tensor_tensor(t2[:], n1b[1][hc][:], eRLm[:],
                                            op=OP.mult)
                    nc.vector.tensor_tensor(l_[:], l_[:], t2[:], op=OP.add)
                    mL.append(l_)
                    r_ = apool.tile([128, 512], BF16, tag=f"mR{hc}", name=f"mR{hc}")
                    nc.vector.tensor_tensor(r_[:], n1b[0][hc][:], eLRm[:],
                                            op=OP.mult)
                    nc.vector.tensor_tensor(t2[:], n1b[1][hc][:], eRRm[:],
                                            op=OP.mult)
                    nc.vector.tensor_tensor(r_[:], r_[:], t2[:], op=OP.add)
                    mR.append(r_)
                if DEBUG_DUMP:
                    for side in (0, 1):
                        for jc2 in range(4):
                            nc.sync.dma_start(
                                dn1_d[:, (side * 4 + jc2) * 512:
                                      (side * 4 + jc2 + 1) * 512],
                                n1b[side][jc2][:])
                    for i, t in enumerate(mA + mL + mR):
                        nc.sync.dma_start(dprod_d[:, i * 512:(i + 1) * 512],
                                          t[:])

                # count row = total expansions (engines can only address
                # partition offsets 0/32/64/96, so use mask row 0s)
                esb_t = apool.tile([1, 512], BF16, tag="esb", name="esb")
                nc.vector.tensor_tensor(esb_t[:], masks[0][0:1, :],
                                        masks[1][0:1, :], op=OP.add)
                for m in range(2, 7):
                    nc.vector.tensor_tensor(esb_t[:], esb_t[:],
                                            masks[m][0:1, :], op=OP.add)
                # count = 1 + 2*esum; 1/count = Exp(-Ln(count)) on Act
                cnt = apool.tile([1, 512], F32, tag="cnt", name="cnt")
                nc.vector.tensor_scalar(cnt[:], esb_t[:], 2.0, 1.0,
                                        OP.mult, OP.add)
                lncnt = apool.tile([1, 512], F32, tag="lncnt", name="lncnt")
                nc.scalar.activation(lncnt[:], cnt[:], AF.Ln)
                recipr = apool.tile([1, 512], F32R, tag="recipr", name="recipr")
                nc.scalar.activation(recipr[:], lncnt[:], AF.Exp, scale=-1.0)
                rbp = scr.tile([128, 512], F32, tag="s", name="rbp")
                nc.tensor.matmul(rbp[:], ones_f32[0:1, :].bitcast(F32R),
                                 recipr[:], start=True, stop=True)
                rb_sb = apool.tile([128, 512], F32, tag="rb", name="rb")
                nc.scalar.activation(rb_sb[:], rbp[:], AF.Identity)
                if DEBUG_DUMP:
                    nc.sync.dma_start(drow_d[:, 0:512], rb_sb[:])

                # agg accumulation — emitted per-hc so the PE starts on
                # mA[0] while later products are still in flight
                aggp = actx.enter_context(tc.tile_pool(name="aggp", bufs=4,
                                                       space="PSUM"))
                # NOTE: PE accumulation groups must be consecutive — never
                # interleave matmuls of different psum groups.  K=1 matmuls
                # corrupt multi-matmul groups on this toolchain; the bias
                # outer-products go in as a single K=7 matmul instead.
                agg_ps = [aggp.tile([128, 512], F32, tag="agg", name=f"agg{jc}")
                          for jc in range(4)]
                for jc in range(4):
                    ap_ = agg_ps[jc]
                    for hc in range(4):
                        nc.tensor.matmul(ap_[:],
                                         wcsb_sb[hc][:, jc * 128:(jc + 1) * 128],
                                         mA[hc][:], start=(hc == 0), stop=False)
                    for hc in range(4):
                        nc.tensor.matmul(ap_[:],
                                         wab_sb[hc][:, jc * 128:(jc + 1) * 128],
                                         mL[hc][:], start=False, stop=False)
                    for hc in range(4):
                        nc.tensor.matmul(ap_[:],
                                         wbb_sb[hc][:, jc * 128:(jc + 1) * 128],
                                         mR[hc][:], start=False, stop=False)
                    nc.tensor.matmul(ap_[:],
                                     bw7_sb[:, jc * 128:(jc + 1) * 128],
                                     mrows[:], start=False, stop=False)
                    nc.tensor.matmul(ap_[:], identb[:],
                                     h0b[jc][:], start=False, stop=True)
                    if DEBUG_DUMP:
                        dag = apool.tile([128, 512], F32, tag=f"dag{jc}",
                                         name=f"dag{jc}")
                        nc.scalar.activation(dag[:], ap_[:], AF.Identity)
                        nc.sync.dma_start(dagg_d[:, jc * 512:(jc + 1) * 512],
                                          dag[:])
                    nc.vector.tensor_tensor(pooT[jc][:], agg_ps[jc][:],
                                            rb_sb[:], op=OP.mult)
                if DEBUG_DUMP:
                    for jc in range(4):
                        nc.sync.dma_start(dpoo_d[:, jc * 512:(jc + 1) * 512],
                                          pooT[jc][:])

        # ---------------- Phase B ----------------
        with ExitStack() as bctx:
            stp = bctx.enter_context(tc.tile_pool(name="stp", bufs=4))
            mmp = bctx.enter_context(tc.tile_pool(name="mmp", bufs=8,
                                                  space="PSUM"))

            drain_idx = [0]

            def drain(dst_ap, ps_ap):
                # alternate 2:1 DVE:Act — DVE psum->bf16 copies are faster
                if drain_idx[0] % 3 == 2:
                    nc.scalar.activation(dst_ap, ps_ap, AF.Identity)
                else:
                    nc.vector.tensor_copy(dst_ap, ps_ap)
                drain_idx[0] += 1

            with nc.allow_low_precision(reason="bf16 matmul inputs"):
                for w in range(NWIN):
                    wt = win_tiles.pop(w)
                    for pc in range(4):
                        stg = stp.tile([128, VW], BF16, tag="stage",
                                       name=f"stg{w}_{pc}")
                        for s_ in range(NSTR):
                            ps = mmp.tile([128, 500], F32, tag="mm",
                                          name=f"mm{w}_{pc}_{s_}", bufs=8)
                            for hc in range(4):
                                nc.tensor.matmul(
                                    ps[:],
                                    pooT[hc][:, pc * 128:(pc + 1) * 128],
                                    wt[hc][:, s_ * 500:(s_ + 1) * 500],
                                    start=(hc == 0), stop=(hc == 3))
                            drain(stg[:, s_ * 500:(s_ + 1) * 500], ps[:])
                        nc.sync.dma_start(
                            logt_d[pc * 128:(pc + 1) * 128, w * VW:(w + 1) * VW],
                            stg[:])
                    if w + 2 < NWIN:
                        win_tiles[w + 2] = load_win(w + 2)

    nc.compile()
    return nc


def _get_nc():
    if "nc" not in _CACHE:
        _CACHE["nc"] = _build()
    return _CACHE["nc"]


def _prep_inputs(tokens, emb, Wp, bp, Wc, bc, Wg, bg, dep, sib, Wout, bout):
    import ml_dtypes
    BF = ml_dtypes.bfloat16
    f64 = np.float64

    tokens = np.asarray(tokens).astype(np.int64).reshape(-1)
    emb = np.ascontiguousarray(np.asarray(emb, dtype=np.float32))
    Wp = np.asarray(Wp, dtype=f64)
    bp = np.asarray(bp, dtype=f64).reshape(-1)
    Wc = np.asarray(Wc, dtype=f64)
    bc = np.asarray(bc, dtype=f64).reshape(-1)
    Wg = np.asarray(Wg, dtype=f64)
    bg = np.asarray(bg, dtype=f64).reshape(-1)
    dep = np.asarray(dep, dtype=f64)
    sib = np.asarray(sib, dtype=f64)
    Wout = np.asarray(Wout, dtype=np.float32)

    WcL = Wc[:, :H]
    WcR = Wc[:, H:]
    wg = Wg[:, 0]
    wcs = WcL + WcR
    biasL = bc[:H] + SIB_SCALE * sib[0]
    biasR = bc[H:] + SIB_SCALE * sib[1]
    bsum = biasL + biasR

    # folded gate vectors (embedding space) + thresholds
    wgY = {0: WcL @ wg, 1: WcR @ wg}
    zvecs = [Wp @ wg, Wp @ wgY[0], Wp @ wgY[1]]
    thr = [-(bp @ wg + DEPTH_EMBED_SCALE * dep[0] @ wg + bg[0])]
    for X, bX in ((0, biasL), (1, biasR)):
        thr.append(-(bp @ wgY[X] + bX @ wg
                     + DEPTH_EMBED_SCALE * dep[1] @ wg + bg[0]))
    for X, bX in ((0, biasL), (1, biasR)):
        WcX = WcL if X == 0 else WcR
        for Y, bY in ((0, biasL), (1, biasR)):
            v = WcX @ wgY[Y]
            zvecs.append(Wp @ v)
            thr.append(-(bp @ v + bX @ wgY[Y] + bY @ wg
                         + DEPTH_EMBED_SCALE * dep[2] @ wg + bg[0]))
    # zvecs order: [root, L, R, LL, LR, RL, RR]; pad col 7 with zeros
    Wz = np.zeros((E, 8), f64)
    for i, v in enumerate(zvecs):
        Wz[:, i] = v
    wz = np.ascontiguousarray(
        Wz.reshape(4, 128, 8).transpose(1, 0, 2).reshape(128, 32)
    ).astype(np.float32)
    thr7 = np.zeros((8, 1), f64)
    thr7[:7, 0] = thr
    thr7[7, 0] = 1e30
    thr7 = thr7.astype(np.float32)

    # AND selector columns (0/1), replicated across the 128 out columns
    selb = np.zeros((7, 7 * 128), np.float32)
    for m, sel in enumerate(MASK_SEL):
        for g in sel:
            selb[g, m * 128:(m + 1) * 128] = 1.0
    selb = selb.astype(BF)
    # narrow selector for the stacked-mask-rows matmul, + count thresholds
    sel7 = np.zeros((7, 8), np.float32)
    for m, sel in enumerate(MASK_SEL):
        for g in sel:
            sel7[g, m] = 1.0
    sel7 = sel7.astype(BF)
    cthr = np.full((8, 1), 1e30, np.float32)
    for m, sel in enumerate(MASK_SEL):
        cthr[m, 0] = len(sel) - 0.5

    # per-partition bias columns for act-engine drains (bp, biasL, biasR)
    cols = np.ascontiguousarray(np.concatenate(
        [bp.reshape(4, 128).T, biasL.reshape(4, 128).T, biasR.reshape(4, 128).T],
        axis=1)).astype(np.float32)

    # agg weights (bf16): wcs, WcL@wcs, WcR@wcs; bias rows b3t [3, H]
    wcsb = wcs.astype(np.float32).astype(BF)
    wab = (WcL @ wcs).astype(np.float32).astype(BF)
    wbb = (WcR @ wcs).astype(np.float32).astype(BF)
    # per-mask bias vectors for the K=7 bias matmul:
    # B[j,p] = sum_m bw7[m,j] * mask_m[p]
    cL = biasL @ wcs
    cR = biasR @ wcs
    bw7 = np.zeros((7, H), f64)
    for m in range(7):
        bw7[m] = bsum
        if m in (3, 5):      # eLL, eRL expand a left child
            bw7[m] += cL
        if m in (4, 6):      # eLR, eRR expand a right child
            bw7[m] += cR
    bw7 = bw7.astype(np.float32).astype(BF)

    wpb = np.ascontiguousarray(Wp).astype(np.float32).astype(BF)
    wcb = np.ascontiguousarray(Wc).astype(np.float32).astype(BF)
    woutb = np.ascontiguousarray(Wout.astype(BF))

    in_maps = []
    for c in range(NCORES):
        tk = tokens[c * PC_POS:(c + 1) * PC_POS]
        g = emb[tk]                                   # [512 pos, 512 E] f32
        hembt = np.ascontiguousarray(
            g.T.reshape(4, 128, 512).transpose(1, 0, 2).reshape(128, 4 * 512))
        in_maps.append({
            "wz": wz, "thr7": thr7, "selb": np.ascontiguousarray(selb),
            "sel7": np.ascontiguousarray(sel7), "cthr": cthr,
            "bw7": np.ascontiguousarray(bw7),
            "cols": cols,
            "hembt": hembt, "hembtb": hembt.astype(BF),
            "wpb": wpb, "wcb": wcb,
            "wcsb": wcsb, "wab": wab, "wbb": wbb,
            "woutb": woutb,
        })
    return in_maps


def _assemble(res, bout=None):
    parts = [np.asarray(res.results[c]["logt"]).astype(np.float32)
             for c in range(NCORES)]
    full = np.concatenate(parts, axis=0)        # [NPOS, V]
    if bout is not None:
        full += np.asarray(bout, dtype=np.float32).reshape(1, V)
    return full.reshape(B, S, V)


def _enable_ldw_opt_once():
    return


def kernel(**inputs) -> np.ndarray:
    from concourse.bass_utils import run_bass_kernel_spmd
    nc = _get_nc()
    in_maps = _prep_inputs(**inputs)
    res = run_bass_kernel_spmd(nc, in_maps, list(range(NCORES)))
    return _assemble(res, bout=inputs["bout"])
